# revision 44
# baseline (speedup 1.0000x reference)
"""LocallyConnected2d (64,64,32,32) x (1,64,64,32,32,9) -> (64,64,32,32) on 8 trn2 cores.

Strategy
--------
Spatial sharding over output rows: core i computes output rows [4i, 4i+4).

Per output location (x, y) the op is an independent GEMM:
    out[:, :, x, y] = patches(x,y) @ W(x,y).T + bias(:, x, y)
with contraction over (c, k) = 64*9 = 576, M = 64 out-channels, N = 64 batch.

On this stack every matmul pays a fresh LDWEIGHTS whose cost is the number of
weight COLUMNS / 1.2 GHz (no fast-weight-load), so the kernel minimizes total
loaded weight columns: 10 matmuls per horizontal location pair (A, B), all
M=64, eight of them K=128 contracting TWO taps at once:

  x band lives in SBUF twice (both with layout (h, w, b), b innermost):
    region1 [128, 13056]: partitions 0-63 plain, 64-127 shifted +1 COLUMN
    region2 [128,  8704]: partitions 0-63 plain, 64-127 shifted +1 ROW
  For A=(x,y) (psum rows 0-63) and B=(x,y+1) (psum rows 64-127):
    A1/A2/A3: region1 cells (x+kh, y),   kh=0..2 -> taps (kh,0)+(kh,1), K=128
    A4:       region2 cell  (x,    y+2)          -> taps (0,2)+(1,2),   K=128
    A5:       region1 lower (x+2,  y+2)          -> tap  (2,2),         K=64
    B1/B2/B3: region1 cells (x+kh, y+1)          -> taps (kh,0)+(kh,1), K=128
    B4:       region2 cell  (x,    y+3)          -> taps (0,2)+(1,2),   K=128
    B5:       region1 UPPER (x+2,  y+2) (=(x+2,y+3)) -> tap (2,2),      K=64
  640 loaded columns per pair vs 768 for the naive schedule; the weight tile
  stays a dense [128, 576] block per pair (A5/B5 share columns across
  partition halves), so HBM weight traffic is unchanged.

Weights are host-prepacked to the exact [K, M] SBUF layout and streamed in 8
tiles of 8 pairs; bias is folded in with one K=8 indicator matmul per PSUM
bank; outputs are copied PSUM->SBUF as fp16 (host casts back to fp32).
x streams in 3 row-pair chunks so the first matmuls start early; per tile the
matmuls needing band rows x..x+1 are emitted before those needing row x+2.

Compute dtype fp16 (fp32 accumulate in PSUM).
"""

import numpy as np

N_B, C, H, W_W, O = 64, 64, 32, 32, 64
KH = KW = 3
NCORES = 8
RPC = H // NCORES            # 4 output rows per core
BAND = RPC + 2               # 6 padded input rows per core
WP = W_W + 2                 # 34 padded width
ROWELEMS = WP * N_B          # 2176 elements per band row
XFREE = BAND * ROWELEMS      # 13056, layout (h, w, b) -- b innermost
X2ROWS = 4                   # region2 holds band rows 0..3 (+1-row shifted up top)
X2FREE = X2ROWS * ROWELEMS   # 8704
NPAIR_CORE = RPC * W_W // 2  # 64 location pairs per core
NTILE = 8                    # PSUM tiles per core (8 pairs each)
PAIR_COLS = 576              # weight cols per location pair
W_FREE = NPAIR_CORE * PAIR_COLS  # 36864

COMPUTE_NP = np.float16      # np.float16 | np.float32 | ml_dtypes.bfloat16
OUT_NP = np.float16          # device output dtype (host casts to fp32)

_CACHE = {}


def _mybir_dt(np_dt):
    import concourse.mybir as mybir
    import ml_dtypes

    if np_dt == np.float16:
        return mybir.dt.float16
    if np_dt == np.float32:
        return mybir.dt.float32
    if np_dt == ml_dtypes.bfloat16:
        return mybir.dt.bfloat16
    raise ValueError(np_dt)


def build_nc(compute_np=None):
    """Build the (single-program) Bass kernel; same NEFF runs on all 8 cores."""
    import concourse.bass as bass  # noqa: F401
    import concourse.mybir as mybir
    import concourse.tile as tile
    from concourse import bacc
    from contextlib import ExitStack

    cdt = _mybir_dt(compute_np or COMPUTE_NP)
    odt = _mybir_dt(OUT_NP)
    f32 = mybir.dt.float32

    nc = bacc.Bacc("TRN2", target_bir_lowering=False, debug=False)

    x_dram = nc.dram_tensor("xb", [128, XFREE // 2], cdt, kind="ExternalInput")
    w_dram = nc.dram_tensor("wp", [128, W_FREE], cdt, kind="ExternalInput")
    # bias columns, the indicator matrix and tile-7's second-half bias
    # (repacked to partitions 0-3) travel as ONE small DMA
    b_dram = nc.dram_tensor("bp", [8, NTILE * 128 + 512 + 128], cdt, kind="ExternalInput")
    o_dram = nc.dram_tensor("out", [NTILE, 128, 512], odt, kind="ExternalOutput")

    with ExitStack() as ctx:
        tc = ctx.enter_context(tile.TileContext(nc))
        const = ctx.enter_context(tc.tile_pool(name="const", bufs=1))
        wpool = ctx.enter_context(tc.tile_pool(name="wpool", bufs=8))
        ppool = ctx.enter_context(tc.tile_pool(name="ppool", bufs=6, space="PSUM"))
        spool = ctx.enter_context(tc.tile_pool(name="spool", bufs=4))

        x1 = const.tile([128, XFREE], cdt)   # [plain | +1 col]
        x2 = const.tile([128, X2FREE], cdt)  # [plain | +1 row]
        bi_sb = const.tile([8, NTILE * 128 + 512 + 128], cdt)

        # tiny, first on scalar so the bias warm-up matmuls start immediately
        nc.scalar.dma_start(bi_sb[:], b_dram.ap()[:, :])

        # x free layout: f = (h*34 + w)*64 + b. The dram tensor is [128, 6528]
        # (band rows 0-2 on partitions 0-63, rows 3-5 on 64-127) so the load
        # engages all 16 SBUF ports; it lands in x1[:, 0:6528] and the vector
        # engine then (a) relocates rows 3-5 to x1[0:64, 6528:], (b) builds
        # the +1-column copy on x1[64:128], (c) builds region2's two halves.
        RE = ROWELEMS
        XH = 3 * RE  # 6528
        cp = nc.vector.tensor_copy
        gp = nc.vector.tensor_copy
        for ch in range(3):
            lo, hi = ch * RE, (ch + 1) * RE
            nc.scalar.dma_start(x1[:, lo:hi], x_dram.ap()[:, lo:hi])
        # dependency-ordered copy pipelines (engine queues are strict FIFO):
        # vector does the row relocations + the +1-col upper copies (each
        # skips the row's last 64 elements -- cell (r, 33) is never read
        # shifted); gpsimd builds region2.
        # NOTE: "relocate row r+3" reads the staged upper of its chunk, which
        # "upper row r" overwrites -- the relocation MUST come first.
        cp(x1[0:64, XH : XH + RE], x1[64:128, 0:RE])              # relocate row 3
        cp(x1[64:128, 0 : RE - 64], x1[0:64, 64:RE])              # upper row 0
        cp(x1[0:64, XH + RE : XH + 2 * RE], x1[64:128, RE : 2 * RE])   # row 4
        cp(x1[64:128, RE : 2 * RE - 64], x1[0:64, RE + 64 : 2 * RE])   # upper row 1
        cp(x1[0:64, XH + 2 * RE : XFREE], x1[64:128, 2 * RE : XH])     # row 5
        cp(x1[64:128, 2 * RE : XH - 64], x1[0:64, 2 * RE + 64 : XH])   # upper row 2
        gp(x2[64:128, 0:RE], x1[0:64, RE : 2 * RE])               # r2 upper row 0
        gp(x2[0:64, 0 : 2 * RE], x1[0:64, 0 : 2 * RE])            # r2 lower rows 0-1
        # copies only needed from tile 2 on are emitted BETWEEN tile blocks so
        # the per-tile PSUM casts (same vector FIFO) aren't stuck behind them
        late_copies = [
            lambda: (
                gp(x2[64:128, RE : 3 * RE], x1[0:64, 2 * RE : 4 * RE]),  # r2 up 1-2
                cp(x1[64:128, XH : 4 * RE - 64], x1[0:64, XH + 64 : 4 * RE]),  # up3
            ),
            lambda: (
                gp(x2[0:64, 2 * RE : 4 * RE], x1[0:64, 2 * RE : 4 * RE]),  # r2 low 2-3
                cp(x1[64:128, 4 * RE : 5 * RE - 64],
                   x1[0:64, 4 * RE + 64 : 5 * RE]),                       # up4
            ),
            lambda: (
                gp(x2[64:128, 3 * RE : 4 * RE], x1[0:64, 4 * RE : 5 * RE]),  # r2 up 3
                cp(x1[64:128, 5 * RE : XFREE - 64],
                   x1[0:64, 5 * RE + 64 : XFREE]),                        # up5
            ),
        ]

        x4a = x1[:].rearrange("p (h w b) -> p h w b", h=BAND, w=WP)   # [128,6,34,64]
        x4b = x2[:].rearrange("p (h w b) -> p h w b", h=X2ROWS, w=WP) # [128,4,34,64]

        # issue ALL weight DMAs up front (they all fit in SBUF with bufs=8):
        # if they were emitted inside the tile loop, the per-tile output-DMA
        # dispatches (which wait on PE progress) would block later weight
        # dispatches in engine program order. x rides sync early, so odd
        # tiles stream on scalar and the later even tiles follow x on sync.
        wts = []
        for t in range(NTILE):
            wt = wpool.tile([128, 8 * PAIR_COLS], cdt)
            wbase = t * 8 * PAIR_COLS
            if t == 0:
                half = 4 * PAIR_COLS
                nc.sync.dma_start(wt[:, 0:half], w_dram.ap()[:, wbase : wbase + half])
                nc.sync.dma_start(
                    wt[:, half : 8 * PAIR_COLS],
                    w_dram.ap()[:, wbase + half : wbase + 8 * PAIR_COLS],
                )
            elif t == NTILE - 1:
                # split w7 too: its second half is the last arrival, so the
                # first 4 pairs can compute while it streams
                half = 4 * PAIR_COLS
                nc.scalar.dma_start(wt[:, 0:half], w_dram.ap()[:, wbase : wbase + half])
                nc.scalar.dma_start(
                    wt[:, half : 8 * PAIR_COLS],
                    w_dram.ap()[:, wbase + half : wbase + 8 * PAIR_COLS],
                )
            else:
                # the scalar (ACT) HWDGE ring drains ~2x slower than sync's,
                # so it only carries the late-needed tiles
                weng = nc.scalar if t in (3, 5) else nc.sync
                weng.dma_start(wt[:], w_dram.ap()[:, wbase : wbase + 8 * PAIR_COLS])
            wts.append(wt)

        def emit_bank(t, wt, jplo, jphi):
            """Bias + all matmuls for pairs [jplo, jphi) of tile t into one
            PSUM bank, then cast + output DMA."""
            xh = t // 2
            nj = jphi - jplo
            ps = ppool.tile([128, 64 * nj], f32)
            # bias first: warms the PE while x/weights stream in, and clears
            # the bank (start=True): psum[p, j*64+b] = bias[j, t*128+p]
            if jplo == 0:
                blhs = bi_sb[0:nj, t * 128 : (t + 1) * 128]
            else:
                # repacked section: tile-7 pairs 4-7 on partitions 0-3
                blhs = bi_sb[0:nj, NTILE * 128 + 512 : NTILE * 128 + 512 + 128]
            nc.tensor.matmul(
                ps[:, :],
                blhs,
                bi_sb[0:nj, NTILE * 128 : NTILE * 128 + 64 * nj],
                start=True,
                stop=False,
                skip_group_check=True,
            )
            mm = nc.tensor.matmul
            # group 1: needs band rows xh, xh+1 only
            for jp in range(jplo, jphi):
                yA = 2 * ((t % 2) * 8 + jp)
                base = jp * PAIR_COLS
                oc = (jp - jplo) * 64
                # A1/B1: region1 cells (xh, yA/yA+1): taps (0,0)+(0,1)
                mm(ps[0:64, oc : oc + 64], wt[:, base : base + 64],
                   x4a[:, xh, yA, :], start=False, stop=False, skip_group_check=True)
                mm(ps[64:128, oc : oc + 64], wt[:, base + 256 : base + 320],
                   x4a[:, xh, yA + 1, :], start=False, stop=False, skip_group_check=True)
                # A2/B2: region1 cells (xh+1, .): taps (1,0)+(1,1)
                mm(ps[0:64, oc : oc + 64], wt[:, base + 64 : base + 128],
                   x4a[:, xh + 1, yA, :], start=False, stop=False, skip_group_check=True)
                mm(ps[64:128, oc : oc + 64], wt[:, base + 320 : base + 384],
                   x4a[:, xh + 1, yA + 1, :], start=False, stop=False, skip_group_check=True)
            # group 2: needs band row xh+2 / region2
            for jp in range(jplo, jphi):
                yA = 2 * ((t % 2) * 8 + jp)
                base = jp * PAIR_COLS
                oc = (jp - jplo) * 64
                # A3/B3: region1 cells (xh+2, .): taps (2,0)+(2,1)
                mm(ps[0:64, oc : oc + 64], wt[:, base + 128 : base + 192],
                   x4a[:, xh + 2, yA, :], start=False, stop=False, skip_group_check=True)
                mm(ps[64:128, oc : oc + 64], wt[:, base + 384 : base + 448],
                   x4a[:, xh + 2, yA + 1, :], start=False, stop=False, skip_group_check=True)
                # A4/B4: region2 cells (xh, yA+2/yA+3): taps (0,2)+(1,2)
                mm(ps[0:64, oc : oc + 64], wt[:, base + 192 : base + 256],
                   x4b[:, xh, yA + 2, :], start=False, stop=False, skip_group_check=True)
                mm(ps[64:128, oc : oc + 64], wt[:, base + 448 : base + 512],
                   x4b[:, xh, yA + 3, :], start=False, stop=False,
                   skip_group_check=True)
            # tap (2,2) singles LAST: their 64x64 row/col tiles block the
            # LDWEIGHTS pull-ahead of any following full-row matmul, so
            # keeping them out of the K=128 stream saves ~180 ns per pair.
            # B5 reads the +1-col copy so both land on cell (xh+2, yA+2).
            for jp in range(jplo, jphi):
                yA = 2 * ((t % 2) * 8 + jp)
                base = jp * PAIR_COLS
                oc = (jp - jplo) * 64
                mm(ps[0:64, oc : oc + 64], wt[0:64, base + 512 : base + 576],
                   x4a[0:64, xh + 2, yA + 2, :], start=False, stop=False,
                   skip_group_check=True)
                mm(ps[64:128, oc : oc + 64], wt[64:128, base + 512 : base + 576],
                   x4a[64:128, xh + 2, yA + 2, :], start=False, stop=(jp == jphi - 1),
                   skip_group_check=True)
            stg = spool.tile([128, 64 * nj], odt, tag="stg")
            nc.vector.tensor_copy(stg[:], ps[:])
            oeng = nc.sync if t % 2 == 0 or t == NTILE - 1 else nc.scalar
            dst = o_dram.ap()[t]
            if nj != 8:
                dst = dst[:, jplo * 64 : jphi * 64]
            oeng.dma_start(dst, stg[:])

        for t in range(NTILE):
            if t == NTILE - 1:
                # split the last tile so its first half's cast/output overlap
                # the second half's matmuls (which wait on the final w7 bytes)
                emit_bank(t, wts[t], 0, 4)
                emit_bank(t, wts[t], 4, 8)
            else:
                emit_bank(t, wts[t], 0, 8)
            if t < len(late_copies):
                late_copies[t]()

    nc.compile()
    return nc


def pack_inputs(x, weight, bias, compute_np=None):
    """Full fp32 inputs -> list of 8 per-core input dicts (device layouts)."""
    cnp = compute_np or COMPUTE_NP
    x = np.asarray(x)
    w5 = np.asarray(weight)[0]        # (o, c, x, y, k)
    b3 = np.asarray(bias)[0]          # (o, x, y)

    xp = np.pad(x, ((0, 0), (0, 0), (1, 1), (1, 1)))  # (b, c, 34, 34)

    ind = np.zeros((8, 512), dtype=cnp)
    for j in range(8):
        ind[j, j * 64 : (j + 1) * 64] = 1.0

    in_maps = []
    for i in range(NCORES):
        band = xp[:, :, RPC * i : RPC * i + BAND, :]          # (b, c, 6, 34)
        xb = np.ascontiguousarray(band.transpose(1, 2, 3, 0)) # (c, 6, 34, b)
        xb = xb.astype(cnp).reshape(64, XFREE)
        # [128, 6528]: band rows 0-2 on partitions 0-63, rows 3-5 on 64-127
        xb = np.concatenate([xb[:, : XFREE // 2], xb[:, XFREE // 2 :]], axis=0)

        wc = w5[:, :, RPC * i : RPC * (i + 1), :, :]          # (o, c, 4, 32, 9)
        # A = even output cols, B = odd; index [o, c, xh, jr, k], k = 3*kh+kw
        A = wc[:, :, :, 0::2, :]
        B = wc[:, :, :, 1::2, :]
        # blocks[xh, jr, part, col]; col layout per pair:
        # [A1 A2 A3 A4 | B1 B2 B3 B4 | S] with 64 cols each; K-halves are the
        # two stacked taps (or A/B for the shared singles block S).
        blk = np.empty((4, 16, 128, PAIR_COLS), dtype=np.float32)

        def put(colsl, khalf, src):  # src[o, c, xh, jr]
            blk[:, :, khalf * 64 : khalf * 64 + 64, colsl] = src.transpose(2, 3, 1, 0)

        for kh in range(3):           # A1-A3 / B1-B3: taps (kh,0)+(kh,1)
            put(slice(64 * kh, 64 * kh + 64), 0, A[..., 3 * kh])
            put(slice(64 * kh, 64 * kh + 64), 1, A[..., 3 * kh + 1])
            put(slice(256 + 64 * kh, 320 + 64 * kh), 0, B[..., 3 * kh])
            put(slice(256 + 64 * kh, 320 + 64 * kh), 1, B[..., 3 * kh + 1])
        # A4/B4: taps (0,2)+(1,2)
        put(slice(192, 256), 0, A[..., 2]); put(slice(192, 256), 1, A[..., 5])
        put(slice(448, 512), 0, B[..., 2]); put(slice(448, 512), 1, B[..., 5])
        # singles: tap (2,2); A on partitions 0-63, B on 64-127
        put(slice(512, 576), 0, A[..., 8]); put(slice(512, 576), 1, B[..., 8])

        # tiles: t = 2*xh + th, pair jp: jr = th*8 + jp
        b6 = blk.reshape(4, 2, 8, 128, PAIR_COLS)
        wp = b6.transpose(3, 0, 1, 2, 4).reshape(128, W_FREE).astype(cnp)

        bc = b3[:, RPC * i : RPC * (i + 1), :]                # (o, 4, 32)
        # bp[jp, t*128 + half*64 + o] = bc[o, xh, 2*(th*8+jp)+half]
        bcr = bc.reshape(64, 4, 2, 8, 2)                      # o xh th jp half
        bp = bcr.transpose(3, 1, 2, 4, 0).reshape(8, NTILE * 128).astype(cnp)
        # + indicator, + tile-7 pairs 4-7 bias repacked to partitions 0-3
        extra = np.zeros((8, 128), dtype=cnp)
        extra[0:4, :] = bp[4:8, 7 * 128 : 8 * 128]
        bp = np.concatenate([bp, ind, extra], axis=1)

        in_maps.append(
            {
                "xb": np.ascontiguousarray(xb),
                "wp": np.ascontiguousarray(wp),
                "bp": np.ascontiguousarray(bp),
            }
        )
    return in_maps


def unpack_output(core_outs):
    """8 per-core [NTILE,128,512] arrays -> full (64, 64, 32, 32) output."""
    arr = np.stack(core_outs)                     # (core, t, part, col)
    arr = arr.reshape(8, 4, 2, 2, 64, 8, 64)      # core xh th half o jp b
    out = arr.transpose(6, 4, 0, 1, 2, 5, 3)      # b o core xh th jp half
    return np.ascontiguousarray(
        out.reshape(64, 64, 32, 32), dtype=np.float32
    )


def run_on_device(in_maps, trace=False, compute_np=None, **kwargs):
    from concourse import bass_utils

    key = ("nc", np.dtype(compute_np or COMPUTE_NP).name)
    if key not in _CACHE:
        _CACHE[key] = build_nc(compute_np)
    nc = _CACHE[key]
    res = bass_utils.run_bass_kernel_spmd(
        nc, in_maps, core_ids=list(range(NCORES)), trace=trace, **kwargs
    )
    return res


def kernel(x, weight, bias):
    in_maps = pack_inputs(x, weight, bias)
    res = run_on_device(in_maps)
    return unpack_output([r["out"] for r in res.results])


# revision 45
# speedup vs baseline: 1.0103x; 1.0103x over previous
"""LocallyConnected2d (64,64,32,32) x (1,64,64,32,32,9) -> (64,64,32,32) on 8 trn2 cores.

Strategy
--------
Spatial sharding over output rows: core i computes output rows [4i, 4i+4).

Per output location (x, y) the op is an independent GEMM:
    out[:, :, x, y] = patches(x,y) @ W(x,y).T + bias(:, x, y)
with contraction over (c, k) = 64*9 = 576, M = 64 out-channels, N = 64 batch.

On this stack every matmul pays a fresh LDWEIGHTS whose cost is the number of
weight COLUMNS / 1.2 GHz (no fast-weight-load), so the kernel minimizes total
loaded weight columns: 10 matmuls per horizontal location pair (A, B), all
M=64, eight of them K=128 contracting TWO taps at once:

  x band lives in SBUF twice (both with layout (h, w, b), b innermost):
    region1 [128, 13056]: partitions 0-63 plain, 64-127 shifted +1 COLUMN
    region2 [128,  8704]: partitions 0-63 plain, 64-127 shifted +1 ROW
  For A=(x,y) (psum rows 0-63) and B=(x,y+1) (psum rows 64-127):
    A1/A2/A3: region1 cells (x+kh, y),   kh=0..2 -> taps (kh,0)+(kh,1), K=128
    A4:       region2 cell  (x,    y+2)          -> taps (0,2)+(1,2),   K=128
    A5:       region1 lower (x+2,  y+2)          -> tap  (2,2),         K=64
    B1/B2/B3: region1 cells (x+kh, y+1)          -> taps (kh,0)+(kh,1), K=128
    B4:       region2 cell  (x,    y+3)          -> taps (0,2)+(1,2),   K=128
    B5:       region1 UPPER (x+2,  y+2) (=(x+2,y+3)) -> tap (2,2),      K=64
  640 loaded columns per pair vs 768 for the naive schedule; the weight tile
  stays a dense [128, 576] block per pair (A5/B5 share columns across
  partition halves), so HBM weight traffic is unchanged.

Weights are host-prepacked to the exact [K, M] SBUF layout and streamed in 8
tiles of 8 pairs; bias is folded in with one K=8 indicator matmul per PSUM
bank; outputs are copied PSUM->SBUF as fp16 (host casts back to fp32).
x streams in 3 row-pair chunks so the first matmuls start early; per tile the
matmuls needing band rows x..x+1 are emitted before those needing row x+2.

Compute dtype fp16 (fp32 accumulate in PSUM).
"""

import numpy as np

N_B, C, H, W_W, O = 64, 64, 32, 32, 64
KH = KW = 3
NCORES = 8
RPC = H // NCORES            # 4 output rows per core
BAND = RPC + 2               # 6 padded input rows per core
WP = W_W + 2                 # 34 padded width
ROWELEMS = WP * N_B          # 2176 elements per band row
XFREE = BAND * ROWELEMS      # 13056, layout (h, w, b) -- b innermost
X2ROWS = 4                   # region2 holds band rows 0..3 (+1-row shifted up top)
X2FREE = X2ROWS * ROWELEMS   # 8704
NPAIR_CORE = RPC * W_W // 2  # 64 location pairs per core
NTILE = 8                    # PSUM tiles per core (8 pairs each)
PAIR_COLS = 576              # weight cols per location pair
W_FREE = NPAIR_CORE * PAIR_COLS  # 36864

COMPUTE_NP = np.float16      # np.float16 | np.float32 | ml_dtypes.bfloat16
OUT_NP = np.float16          # device output dtype (host casts to fp32)

_CACHE = {}


def _mybir_dt(np_dt):
    import concourse.mybir as mybir
    import ml_dtypes

    if np_dt == np.float16:
        return mybir.dt.float16
    if np_dt == np.float32:
        return mybir.dt.float32
    if np_dt == ml_dtypes.bfloat16:
        return mybir.dt.bfloat16
    raise ValueError(np_dt)


def build_nc(compute_np=None):
    """Build the (single-program) Bass kernel; same NEFF runs on all 8 cores."""
    import concourse.bass as bass  # noqa: F401
    import concourse.mybir as mybir
    import concourse.tile as tile
    from concourse import bacc
    from contextlib import ExitStack

    cdt = _mybir_dt(compute_np or COMPUTE_NP)
    odt = _mybir_dt(OUT_NP)
    f32 = mybir.dt.float32

    nc = bacc.Bacc("TRN2", target_bir_lowering=False, debug=False)

    x_dram = nc.dram_tensor("xb", [128, XFREE // 2], cdt, kind="ExternalInput")
    w_dram = nc.dram_tensor("wp", [128, W_FREE], cdt, kind="ExternalInput")
    # bias columns and the indicator matrix travel as ONE small DMA
    b_dram = nc.dram_tensor("bp", [8, NTILE * 128 + 512], cdt, kind="ExternalInput")
    o_dram = nc.dram_tensor("out", [NTILE, 128, 512], odt, kind="ExternalOutput")

    with ExitStack() as ctx:
        tc = ctx.enter_context(tile.TileContext(nc))
        const = ctx.enter_context(tc.tile_pool(name="const", bufs=1))
        wpool = ctx.enter_context(tc.tile_pool(name="wpool", bufs=8))
        ppool = ctx.enter_context(tc.tile_pool(name="ppool", bufs=6, space="PSUM"))
        spool = ctx.enter_context(tc.tile_pool(name="spool", bufs=4))

        x1 = const.tile([128, XFREE], cdt)   # [plain | +1 col]
        x2 = const.tile([128, X2FREE], cdt)  # [plain | +1 row]
        bi_sb = const.tile([8, NTILE * 128 + 512], cdt)

        # tiny, first on scalar so the bias warm-up matmuls start immediately
        nc.scalar.dma_start(bi_sb[:], b_dram.ap()[:, :])

        # x free layout: f = (h*34 + w)*64 + b. The dram tensor is [128, 6528]
        # (band rows 0-2 on partitions 0-63, rows 3-5 on 64-127) so the load
        # engages all 16 SBUF ports; it lands in x1[:, 0:6528] and the vector
        # engine then (a) relocates rows 3-5 to x1[0:64, 6528:], (b) builds
        # the +1-column copy on x1[64:128], (c) builds region2's two halves.
        RE = ROWELEMS
        XH = 3 * RE  # 6528
        cp = nc.vector.tensor_copy
        gp = nc.vector.tensor_copy
        for ch in range(3):
            lo, hi = ch * RE, (ch + 1) * RE
            nc.scalar.dma_start(x1[:, lo:hi], x_dram.ap()[:, lo:hi])
        # dependency-ordered copy pipelines (engine queues are strict FIFO):
        # vector does the row relocations + the +1-col upper copies (each
        # skips the row's last 64 elements -- cell (r, 33) is never read
        # shifted); gpsimd builds region2.
        # NOTE: "relocate row r+3" reads the staged upper of its chunk, which
        # "upper row r" overwrites -- the relocation MUST come first.
        cp(x1[0:64, XH : XH + RE], x1[64:128, 0:RE])              # relocate row 3
        cp(x1[64:128, 0 : RE - 64], x1[0:64, 64:RE])              # upper row 0
        cp(x1[0:64, XH + RE : XH + 2 * RE], x1[64:128, RE : 2 * RE])   # row 4
        cp(x1[64:128, RE : 2 * RE - 64], x1[0:64, RE + 64 : 2 * RE])   # upper row 1
        cp(x1[0:64, XH + 2 * RE : XFREE], x1[64:128, 2 * RE : XH])     # row 5
        cp(x1[64:128, 2 * RE : XH - 64], x1[0:64, 2 * RE + 64 : XH])   # upper row 2
        gp(x2[64:128, 0:RE], x1[0:64, RE : 2 * RE])               # r2 upper row 0
        gp(x2[0:64, 0 : 2 * RE], x1[0:64, 0 : 2 * RE])            # r2 lower rows 0-1
        # copies only needed from tile 2 on are emitted BETWEEN tile blocks so
        # the per-tile PSUM casts (same vector FIFO) aren't stuck behind them
        late_copies = [
            lambda: (
                gp(x2[64:128, RE : 3 * RE], x1[0:64, 2 * RE : 4 * RE]),  # r2 up 1-2
                cp(x1[64:128, XH : 4 * RE - 64], x1[0:64, XH + 64 : 4 * RE]),  # up3
            ),
            lambda: (
                gp(x2[0:64, 2 * RE : 4 * RE], x1[0:64, 2 * RE : 4 * RE]),  # r2 low 2-3
                cp(x1[64:128, 4 * RE : 5 * RE - 64],
                   x1[0:64, 4 * RE + 64 : 5 * RE]),                       # up4
            ),
            lambda: (
                gp(x2[64:128, 3 * RE : 4 * RE], x1[0:64, 4 * RE : 5 * RE]),  # r2 up 3
                cp(x1[64:128, 5 * RE : XFREE - 64],
                   x1[0:64, 5 * RE + 64 : XFREE]),                        # up5
            ),
        ]

        x4a = x1[:].rearrange("p (h w b) -> p h w b", h=BAND, w=WP)   # [128,6,34,64]
        x4b = x2[:].rearrange("p (h w b) -> p h w b", h=X2ROWS, w=WP) # [128,4,34,64]

        # issue ALL weight DMAs up front (they all fit in SBUF with bufs=8):
        # if they were emitted inside the tile loop, the per-tile output-DMA
        # dispatches (which wait on PE progress) would block later weight
        # dispatches in engine program order. x rides sync early, so odd
        # tiles stream on scalar and the later even tiles follow x on sync.
        wts = []
        for t in range(NTILE):
            wt = wpool.tile([128, 8 * PAIR_COLS], cdt)
            wbase = t * 8 * PAIR_COLS
            if t == 0:
                half = 4 * PAIR_COLS
                nc.sync.dma_start(wt[:, 0:half], w_dram.ap()[:, wbase : wbase + half])
                nc.sync.dma_start(
                    wt[:, half : 8 * PAIR_COLS],
                    w_dram.ap()[:, wbase + half : wbase + 8 * PAIR_COLS],
                )
            else:
                # the scalar (ACT) HWDGE ring drains ~2x slower than sync's,
                # so it only carries the late-needed tiles
                weng = nc.scalar if t in (3, 5, 7) else nc.sync
                weng.dma_start(wt[:], w_dram.ap()[:, wbase : wbase + 8 * PAIR_COLS])
            wts.append(wt)

        for t in range(NTILE):
            wt = wts[t]
            ps = ppool.tile([128, 512], f32)
            xh = t // 2
            # bias first: warms the PE while x/weights stream in, and clears
            # the bank (start=True): psum[p, j*64+b] = bias[j, t*128+p]
            nc.tensor.matmul(
                ps[:, :],
                bi_sb[:, t * 128 : (t + 1) * 128],
                bi_sb[:, NTILE * 128 : NTILE * 128 + 512],
                start=True,
                stop=False,
                skip_group_check=True,
            )
            # group 1: needs band rows xh, xh+1 only
            for jp in range(8):
                yA = 2 * ((t % 2) * 8 + jp)
                base = jp * PAIR_COLS
                oc = jp * 64
                mm = nc.tensor.matmul
                # A1/B1: region1 cells (xh, yA/yA+1): taps (0,0)+(0,1)
                mm(ps[0:64, oc : oc + 64], wt[:, base : base + 64],
                   x4a[:, xh, yA, :], start=False, stop=False, skip_group_check=True)
                mm(ps[64:128, oc : oc + 64], wt[:, base + 256 : base + 320],
                   x4a[:, xh, yA + 1, :], start=False, stop=False, skip_group_check=True)
                # A2/B2: region1 cells (xh+1, .): taps (1,0)+(1,1)
                mm(ps[0:64, oc : oc + 64], wt[:, base + 64 : base + 128],
                   x4a[:, xh + 1, yA, :], start=False, stop=False, skip_group_check=True)
                mm(ps[64:128, oc : oc + 64], wt[:, base + 320 : base + 384],
                   x4a[:, xh + 1, yA + 1, :], start=False, stop=False, skip_group_check=True)
            # group 2: needs band row xh+2 / region2
            for jp in range(8):
                yA = 2 * ((t % 2) * 8 + jp)
                base = jp * PAIR_COLS
                oc = jp * 64
                mm = nc.tensor.matmul
                # A3/B3: region1 cells (xh+2, .): taps (2,0)+(2,1)
                mm(ps[0:64, oc : oc + 64], wt[:, base + 128 : base + 192],
                   x4a[:, xh + 2, yA, :], start=False, stop=False, skip_group_check=True)
                mm(ps[64:128, oc : oc + 64], wt[:, base + 384 : base + 448],
                   x4a[:, xh + 2, yA + 1, :], start=False, stop=False, skip_group_check=True)
                # A4/B4: region2 cells (xh, yA+2/yA+3): taps (0,2)+(1,2)
                mm(ps[0:64, oc : oc + 64], wt[:, base + 192 : base + 256],
                   x4b[:, xh, yA + 2, :], start=False, stop=False, skip_group_check=True)
                mm(ps[64:128, oc : oc + 64], wt[:, base + 448 : base + 512],
                   x4b[:, xh, yA + 3, :], start=False, stop=False,
                   skip_group_check=True)
            # tap (2,2) singles LAST: their 64x64 row/col tiles block the
            # LDWEIGHTS pull-ahead of any following full-row matmul, so
            # keeping them out of the K=128 stream saves ~180 ns per pair.
            # B5 reads the +1-col copy so both land on cell (xh+2, yA+2).
            for jp in range(8):
                yA = 2 * ((t % 2) * 8 + jp)
                base = jp * PAIR_COLS
                oc = jp * 64
                mm = nc.tensor.matmul
                mm(ps[0:64, oc : oc + 64], wt[0:64, base + 512 : base + 576],
                   x4a[0:64, xh + 2, yA + 2, :], start=False, stop=False,
                   skip_group_check=True)
                mm(ps[64:128, oc : oc + 64], wt[64:128, base + 512 : base + 576],
                   x4a[64:128, xh + 2, yA + 2, :], start=False, stop=(jp == 7),
                   skip_group_check=True)
            stg = spool.tile([128, 512], odt)
            nc.vector.tensor_copy(stg[:], ps[:])
            oeng = nc.sync if t % 2 == 0 or t == NTILE - 1 else nc.scalar
            oeng.dma_start(o_dram.ap()[t], stg[:])
            if t < len(late_copies):
                late_copies[t]()

    nc.compile()
    return nc


def pack_inputs(x, weight, bias, compute_np=None):
    """Full fp32 inputs -> list of 8 per-core input dicts (device layouts)."""
    cnp = compute_np or COMPUTE_NP
    x = np.asarray(x)
    w5 = np.asarray(weight)[0]        # (o, c, x, y, k)
    b3 = np.asarray(bias)[0]          # (o, x, y)

    xp = np.pad(x, ((0, 0), (0, 0), (1, 1), (1, 1)))  # (b, c, 34, 34)

    ind = np.zeros((8, 512), dtype=cnp)
    for j in range(8):
        ind[j, j * 64 : (j + 1) * 64] = 1.0

    in_maps = []
    for i in range(NCORES):
        band = xp[:, :, RPC * i : RPC * i + BAND, :]          # (b, c, 6, 34)
        xb = np.ascontiguousarray(band.transpose(1, 2, 3, 0)) # (c, 6, 34, b)
        xb = xb.astype(cnp).reshape(64, XFREE)
        # [128, 6528]: band rows 0-2 on partitions 0-63, rows 3-5 on 64-127
        xb = np.concatenate([xb[:, : XFREE // 2], xb[:, XFREE // 2 :]], axis=0)

        wc = w5[:, :, RPC * i : RPC * (i + 1), :, :]          # (o, c, 4, 32, 9)
        # A = even output cols, B = odd; index [o, c, xh, jr, k], k = 3*kh+kw
        A = wc[:, :, :, 0::2, :]
        B = wc[:, :, :, 1::2, :]
        # blocks[xh, jr, part, col]; col layout per pair:
        # [A1 A2 A3 A4 | B1 B2 B3 B4 | S] with 64 cols each; K-halves are the
        # two stacked taps (or A/B for the shared singles block S).
        blk = np.empty((4, 16, 128, PAIR_COLS), dtype=np.float32)

        def put(colsl, khalf, src):  # src[o, c, xh, jr]
            blk[:, :, khalf * 64 : khalf * 64 + 64, colsl] = src.transpose(2, 3, 1, 0)

        for kh in range(3):           # A1-A3 / B1-B3: taps (kh,0)+(kh,1)
            put(slice(64 * kh, 64 * kh + 64), 0, A[..., 3 * kh])
            put(slice(64 * kh, 64 * kh + 64), 1, A[..., 3 * kh + 1])
            put(slice(256 + 64 * kh, 320 + 64 * kh), 0, B[..., 3 * kh])
            put(slice(256 + 64 * kh, 320 + 64 * kh), 1, B[..., 3 * kh + 1])
        # A4/B4: taps (0,2)+(1,2)
        put(slice(192, 256), 0, A[..., 2]); put(slice(192, 256), 1, A[..., 5])
        put(slice(448, 512), 0, B[..., 2]); put(slice(448, 512), 1, B[..., 5])
        # singles: tap (2,2); A on partitions 0-63, B on 64-127
        put(slice(512, 576), 0, A[..., 8]); put(slice(512, 576), 1, B[..., 8])

        # tiles: t = 2*xh + th, pair jp: jr = th*8 + jp
        b6 = blk.reshape(4, 2, 8, 128, PAIR_COLS)
        wp = b6.transpose(3, 0, 1, 2, 4).reshape(128, W_FREE).astype(cnp)

        bc = b3[:, RPC * i : RPC * (i + 1), :]                # (o, 4, 32)
        # bp[jp, t*128 + half*64 + o] = bc[o, xh, 2*(th*8+jp)+half]
        bcr = bc.reshape(64, 4, 2, 8, 2)                      # o xh th jp half
        bp = bcr.transpose(3, 1, 2, 4, 0).reshape(8, NTILE * 128).astype(cnp)
        bp = np.concatenate([bp, ind], axis=1)                # + indicator

        in_maps.append(
            {
                "xb": np.ascontiguousarray(xb),
                "wp": np.ascontiguousarray(wp),
                "bp": np.ascontiguousarray(bp),
            }
        )
    return in_maps


def unpack_output(core_outs):
    """8 per-core [NTILE,128,512] arrays -> full (64, 64, 32, 32) output."""
    arr = np.stack(core_outs)                     # (core, t, part, col)
    arr = arr.reshape(8, 4, 2, 2, 64, 8, 64)      # core xh th half o jp b
    out = arr.transpose(6, 4, 0, 1, 2, 5, 3)      # b o core xh th jp half
    return np.ascontiguousarray(
        out.reshape(64, 64, 32, 32), dtype=np.float32
    )


def run_on_device(in_maps, trace=False, compute_np=None, **kwargs):
    from concourse import bass_utils

    key = ("nc", np.dtype(compute_np or COMPUTE_NP).name)
    if key not in _CACHE:
        _CACHE[key] = build_nc(compute_np)
    nc = _CACHE[key]
    res = bass_utils.run_bass_kernel_spmd(
        nc, in_maps, core_ids=list(range(NCORES)), trace=trace, **kwargs
    )
    return res


def kernel(x, weight, bias):
    in_maps = pack_inputs(x, weight, bias)
    res = run_on_device(in_maps)
    return unpack_output([r["out"] for r in res.results])
